# revision 18
# baseline (speedup 1.0000x reference)
"""FAIM head kernel for Trainium2 (8 NeuronCores, SPMD over class shards).

Computes out[b,c] = -scale * (sqrt((x_b-mu_c)^T Sigma (x_b-mu_c) + eps)
                              + lmbda * sqrt((beta.(x_b-mu_c))^2 + eps))
with Sigma = tril(L) @ tril(L)^T + eps*I.

Algebraic decomposition (~5e-3 max rel err on hardware vs the naive
reference, tolerance 2e-2): with Lt = tril(L),
YMT[j] = Lt^T [x^T | mu^T] block-row j:
  quad[b,c] = a[b] + g[c] - 2*cross[b,c]
  a[b]    = |Y_b|^2 ; g[c] = |M_c|^2 ; cross = Y M^T
  beta_dot[b,c] = (x beta)[b] - (mu beta)[c]
The eps terms contribute <1e-5 relative and are dropped entirely.

The dominant YMT matmuls run in fp8e4m3 DoubleRow (two 128-deep k-tiles per
instruction at 0.5 cycles/row). Inputs are pre-transposed/packed on host:
  xmuT [128, 8, 256] fp8  chunk d: cols 0:128 = x^T_d, 128:253 = mu_shard^T_d
  Lt   [128, 4608]   fp8  the 36 lower-tri lhsT blocks, slab j = blocks d>=j
  bcb  [128, 8, 128] fp8  beta broadcast tiles (replicated layout)
so the kernel does no on-device transposes or tril masking. beta_dot is 4
DoubleRow matmuls with the broadcast-beta lhsT; (x beta) is the diagonal of
the broadcast output. The bf16 trail accumulates the Y gram + cross in one
full-width matmul per block (a[b] = gram diagonal, extracted once via an
identity*scale^2 mask + row-reduce) plus a -g/2 column-sum matmul. Squares
split DVE/Pool from the bf16 SBUF copy of Y; PSUM->SBUF copies split
ACT/DVE. Input DMAs are batched into 5 large transfers spread over the
sync/scalar HWDGE + Pool SWDGE queues (trigger instructions cost ~500ns of
issuing-engine time); the output DMA rides Pool SWDGE. Epilogue is fused:
riem = Sqrt(-2s^2*pg_mu + diag) via one activation, |.| via mult+min,
final subtract on Pool.

Per-engine balance (CoreSim, marginal per rep): ~2.8us vs ~11.1us for the
pre-optimization baseline; all of ACT/DVE/PE/Pool/SP land within ~2-3us.

Sharding: classes C=1000 split 125 per core; x/L/beta replicated.
"""

import numpy as np

try:
    import concourse.bass as bass
except ImportError:  # pragma: no cover
    import sys

    sys.path.insert(0, "/opt/trn_rl_repo")
    import concourse.bass as bass

import concourse.bacc as bacc
import concourse.mybir as mybir
import concourse.tile as tile
from concourse.bass_utils import run_bass_kernel_spmd
from concourse.masks import make_identity

F32 = mybir.dt.float32
BF16 = mybir.dt.bfloat16
F8 = mybir.dt.float8e4
B, C, D = 128, 1000, 1024
NCORES = 8
CS = C // NCORES  # 125 classes per core
ND = D // 128  # 8 chunks of 128 along D
W = 128 + CS  # 253 used cols of the [xT | muT] block

SLAB_NB = [ND - j for j in range(ND)]  # blocks per slab j
SLAB_OFF = [0]
for n in SLAB_NB:
    SLAB_OFF.append(SLAB_OFF[-1] + n * 128)
LT_COLS = SLAB_OFF[-1]  # 4608

# group issue order: start with cheap-to-unblock j=6, end with the 1-matmul
# j=7 group so the post-YMT tail (copy+trail+sqrt+out) trails a tiny group.
GRP_ORDER = [6, 5, 4, 3, 2, 1, 0, 7]

MULT = mybir.AluOpType.mult
ADD = mybir.AluOpType.add
SUB = mybir.AluOpType.subtract

_cached_nc = None


def _build(rep=1):
    # rep>1 unrolls the whole body rep times — used only by test.py to
    # measure marginal per-iteration device time; kernel() always uses rep=1.
    nc = bacc.Bacc(
        "TRN2", target_bir_lowering=False, debug=False, num_devices=NCORES
    )
    xmuT_d = nc.dram_tensor("xmuT", [128, ND, 256], F8, kind="ExternalInput")
    Lt_d = nc.dram_tensor("Lt", [128, LT_COLS], F8, kind="ExternalInput")
    # beta broadcast tiles (host-replicated layout): bcb[p,d,m] = beta[d*128+p]
    bcb_d = nc.dram_tensor("bcb", [128, ND, 128], F8, kind="ExternalInput")
    # bsc cols: 0:8 betaT chunks, 8 = lmbda*(-scale), 9 = scale^2, 10 = -2*scale^2
    bsc_d = nc.dram_tensor("bsc", [128, 12], F32, kind="ExternalInput")
    # chain: unused dummy input; lets a timing harness serialize executions
    # device-side by feeding the previous call's output into it (no XLA glue)
    chain_d = nc.dram_tensor("chain", [B, CS], F32, kind="ExternalInput")
    out_d = nc.dram_tensor("out", [B, CS], F32, kind="ExternalOutput")

    def slab(t, j):
        return t[:, SLAB_OFF[j] : SLAB_OFF[j + 1]]

    with tile.TileContext(nc) as tc:
        with (
            tc.tile_pool(name="const", bufs=2) as const,
            tc.tile_pool(name="data", bufs=2) as data,
            tc.tile_pool(name="epi", bufs=2) as epi,
            tc.tile_pool(name="psy", bufs=3, space="PSUM") as psy,
            tc.tile_pool(name="acca", bufs=2, space="PSUM") as acca,
            tc.tile_pool(name="accb", bufs=1, space="PSUM") as accb,
        ):
            chain_sb = const.tile([128, 128], F32, name="chain_sb", tag="chn")
            nc.scalar.dma_start(out=chain_sb[:, 0:CS], in_=chain_d[:])
            # loop-invariant constants, loaded/built once
            ones = const.tile([128, 128], BF16, name="ones", tag="ones")
            nc.vector.memset(ones, 1.0)
            negh = const.tile([128, 128], BF16, name="negh", tag="negh")
            nc.vector.memset(negh, -0.5)
            ident = const.tile([128, 128], BF16, name="ident", tag="ident")
            make_identity(nc, ident)
            ident_s2 = const.tile([128, 128], F32, name="ident_s2", tag="ids2")
            bsc_sb = const.tile([128, 12], F32, name="bsc_sb", tag="bsc")
            nc.scalar.dma_start(out=bsc_sb, in_=bsc_d[:])
            betaT_sb = bsc_sb[:, 0:ND]
            lmn = bsc_sb[:, 8:9]
            s2 = bsc_sb[:, 9:10]
            m2s2 = bsc_sb[:, 10:11]
            nc.vector.tensor_scalar_mul(out=ident_s2, in0=ident, scalar1=s2)
            for _r in range(rep):
                # ---------------- input DMAs ----------------
                # few, large transfers: DMA trigger instructions cost ~500ns
                # on the issuing engine, so batch slabs. xmuT high half +
                # slabs 4-7 unblock the first five groups.
                xm = data.tile([128, ND, 256], F8, name="xm", tag="xm")
                bcb = data.tile([128, ND, 128], F8, name="bcb", tag="bcb")
                Ls = data.tile([128, LT_COLS], F8, name="Ls", tag="Ls")
                nc.sync.dma_start(out=xm[:, 4:8, :], in_=xmuT_d[:, 4:8, :])
                nc.gpsimd.dma_start(out=bcb, in_=bcb_d[:])
                nc.sync.dma_start(
                    out=Ls[:, SLAB_OFF[4] :], in_=Lt_d[:, SLAB_OFF[4] :]
                )
                nc.sync.dma_start(
                    out=Ls[:, SLAB_OFF[2] : SLAB_OFF[4]],
                    in_=Lt_d[:, SLAB_OFF[2] : SLAB_OFF[4]],
                )
                nc.gpsimd.dma_start(out=xm[:, 0:4, :], in_=xmuT_d[:, 0:4, :])
                nc.sync.dma_start(
                    out=Ls[:, 0 : SLAB_OFF[2]], in_=Lt_d[:, 0 : SLAB_OFF[2]]
                )

                # persistent accumulators; pg/pa are read only by the rep
                # epilogue, so double-buffer them for cross-rep overlap
                pg = acca.tile([128, 256], F32, name="pg", tag="pg")
                pbb = accb.tile([128, 256], F32, name="pbb", tag="pbb")

                ym = {}
                ym2 = {}

                def do_group(j, w_mac=None):
                    py = psy.tile([128, 256], F32, name=f"py{j}", tag="py")
                    k = ND - j
                    # fp8 DoubleRow: two adjacent d-blocks per instruction.
                    # pairs are even-d aligned; odd counts leave the diagonal
                    # block d=j as a plain fp8 matmul (issued last).
                    pairs = list(range(j + (k % 2), ND, 2))
                    steps = [("pair", d) for d in reversed(pairs)]
                    if k % 2:
                        steps.append(("single", j))
                    for i, (kind, d) in enumerate(steps):
                        o = SLAB_OFF[j] + (d - j) * 128
                        first = i == 0
                        last = i == len(steps) - 1
                        if kind == "pair":
                            nc.tensor.matmul(
                                py[:, 0:W],
                                lhsT=Ls[:, o : o + 256].rearrange(
                                    "p (two m) -> p two m", two=2
                                ),
                                rhs=xm[:, d : d + 2, 0:W],
                                start=first, stop=last,
                                perf_mode=mybir.MatmulPerfMode.DoubleRow,
                            )
                        else:
                            nc.tensor.matmul(
                                py[:, 0:W], lhsT=Ls[:, o : o + 128],
                                rhs=xm[:, d, 0:W],
                                start=first, stop=last,
                            )
                    y = data.tile([128, 256], BF16, name=f"ym{j}", tag=f"ym{j}")
                    if j in (5, 3, 1):
                        nc.vector.tensor_copy(out=y[:, 0:W], in_=py[:, 0:W])
                    else:
                        nc.scalar.copy(out=y[:, 0:W], in_=py[:, 0:W])
                    y2 = data.tile(
                        [128, 128], BF16, name=f"ym2{j}", tag=f"ym2{j}"
                    )
                    if j in (6, 4, 2, 0):
                        nc.gpsimd.tensor_mul(
                            out=y2[:, 0:CS], in0=y[:, 128:W], in1=y[:, 128:W]
                        )
                    else:
                        nc.vector.tensor_mul(
                            out=y2[:, 0:CS], in0=y[:, 128:W], in1=y[:, 128:W]
                        )
                    ym[j], ym2[j] = y, y2

                first_t = GRP_ORDER[0]
                last_t = GRP_ORDER[-1]

                def do_trail(j):
                    ft = j == first_t
                    lt = j == last_t
                    # full-width: cols 0:128 accumulate the Y gram (diag =
                    # a[b], extracted once at the end), cols 128:253 cross
                    nc.tensor.matmul(
                        pg[:, 0:W], lhsT=ym[j][:, 0:128],
                        rhs=ym[j][:, 0:W],
                        start=ft, stop=False, skip_group_check=True,
                    )
                    # -g[c]/2 broadcast over partitions
                    nc.tensor.matmul(
                        pg[:, 128:W], lhsT=negh, rhs=ym2[j][:, 0:CS],
                        start=False, stop=lt, skip_group_check=True,
                    )

                do_group(6, w_mac=7)
                do_group(5, w_mac=6)
                do_trail(6)
                do_group(4, w_mac=5)
                do_trail(5)
                do_group(3, w_mac=4)
                do_trail(4)
                do_group(2, w_mac=3)
                do_trail(3)
                do_group(1, w_mac=2)
                do_trail(2)
                # beta_dot via 4 fp8 DoubleRow matmuls with broadcast-beta
                # lhsT: pbb rows all equal beta.x_b | beta.mu_c
                for i, d in enumerate((6, 4, 2, 0)):
                    nc.tensor.matmul(
                        pbb[:, 0:W],
                        lhsT=bcb[:, d : d + 2, :],
                        rhs=xm[:, d : d + 2, 0:W],
                        start=(i == 0), stop=(i == 3),
                        perf_mode=mybir.MatmulPerfMode.DoubleRow,
                    )
                # (x beta) column = diagonal of the x-half broadcast block
                atm = epi.tile([128, 128], BF16, name="atm", tag="atm")
                nc.vector.tensor_tensor(
                    out=atm, in0=pbb[:, 0:128], in1=ident, op=MULT,
                )
                bxs = epi.tile([128, 2], F32, name="bxs", tag="bxs")
                nc.vector.tensor_reduce(
                    out=bxs[:, 0:1], in_=atm, axis=mybir.AxisListType.X,
                    op=ADD,
                )
                bd = epi.tile([128, 128], F32, name="bd", tag="bd")
                nc.vector.tensor_scalar(
                    out=bd[:, 0:CS], in0=pbb[:, 128:W], scalar1=bxs[:, 0:1],
                    scalar2=lmn, op0=SUB, op1=MULT,
                )
                do_group(0)
                do_trail(1)
                # negabs = min(-t, t) = -|t| in one scalar_tensor_tensor
                dirabs = epi.tile([128, 128], F32, name="dirabs", tag="dirabs")
                nc.vector.scalar_tensor_tensor(
                    out=dirabs[:, 0:CS], in0=bd[:, 0:CS], scalar=-1.0,
                    in1=bd[:, 0:CS], op0=MULT, op1=mybir.AluOpType.min,
                )
                do_group(7)
                do_trail(0)
                do_trail(7)

                # ---------------- epilogue ----------------
                # riemannian side: riem = scale*sqrt(quad)
                #   = Sqrt(pg_mu * (-2 s^2) + s^2 * a),  a = diag(pg_x)
                am = epi.tile([128, 128], F32, name="am", tag="am")
                nc.vector.tensor_tensor(
                    out=am, in0=pg[:, 0:128], in1=ident_s2, op=MULT,
                )
                a_s2 = epi.tile([128, 2], F32, name="a_s2", tag="a_s2")
                nc.vector.tensor_reduce(
                    out=a_s2[:, 0:1], in_=am, axis=mybir.AxisListType.X,
                    op=ADD,
                )
                riem = epi.tile([128, 128], F32, name="riem", tag="riem")
                nc.scalar.activation(
                    out=riem[:, 0:CS], in_=pg[:, 128:W],
                    func=mybir.ActivationFunctionType.Sqrt,
                    scale=m2s2, bias=a_s2[:, 0:1],
                )
                # out = lmn*|bd| - riem = -scale*(sqrt(quad) + lmbda*|bd|)
                res = epi.tile([128, 128], F32, name="res", tag="res")
                nc.gpsimd.tensor_tensor(
                    out=res[:, 0:CS], in0=dirabs[:, 0:CS], in1=riem[:, 0:CS],
                    op=SUB,
                )
                nc.gpsimd.dma_start(out=out_d[:], in_=res[:, 0:CS])

    nc.compile()
    return nc


def _pack_inputs(x, mu, beta, L, lmbda, scale):
    """Host-side packing: transpose/shard/convert. Returns per-core input maps."""
    BF = mybir.dt.np(BF16)
    E8 = mybir.dt.np(F8)
    x = np.asarray(x, np.float32)
    mu = np.asarray(mu, np.float32)
    beta = np.asarray(beta, np.float32)
    L = np.asarray(L, np.float32)
    Lt = np.tril(L)

    Lt_pack = np.empty((128, LT_COLS), np.float32)
    for j in range(ND):
        sl = Lt[j * 128 :, j * 128 : (j + 1) * 128]
        sl = sl.reshape(ND - j, 128, 128).transpose(1, 0, 2).reshape(128, -1)
        Lt_pack[:, SLAB_OFF[j] : SLAB_OFF[j + 1]] = sl
    Lt_bf = Lt_pack.astype(E8)

    bsc = np.zeros((128, 12), np.float32)
    betaT = beta.reshape(ND, 128).T
    bsc[:, 0:ND] = betaT
    bcb = np.ascontiguousarray(
        np.broadcast_to(betaT[:, :, None], (128, ND, 128))
    ).astype(mybir.dt.np(F8))
    sc = float(scale)
    bsc[:, 8] = -sc * float(lmbda)
    bsc[:, 9] = sc * sc
    bsc[:, 10] = -2.0 * sc * sc

    xT = x.T.reshape(ND, 128, B).transpose(1, 0, 2)  # [128, 8, 128]
    maps = []
    for i in range(NCORES):
        xmuT = np.zeros((128, ND, 256), np.float32)
        xmuT[:, :, 0:128] = xT
        msh = mu[i * CS : (i + 1) * CS]  # [125, 1024]
        xmuT[:, :, 128:W] = msh.T.reshape(ND, 128, CS).transpose(1, 0, 2)
        maps.append(
            {
                "xmuT": xmuT.astype(E8),
                "bcb": bcb,
                "Lt": Lt_bf,
                "bsc": bsc,
                "chain": np.zeros((B, CS), np.float32),
            }
        )
    return maps


def kernel(x, mu, beta, L, lmbda, scale, **kwargs):
    global _cached_nc
    if _cached_nc is None:
        _cached_nc = _build()
    nc = _cached_nc

    in_maps = _pack_inputs(x, mu, beta, L, lmbda, scale)
    res = run_bass_kernel_spmd(nc, in_maps, core_ids=list(range(NCORES)))
    return np.concatenate(
        [res.results[i]["out"] for i in range(NCORES)], axis=1
    )
